# revision 13
# baseline (speedup 1.0000x reference)
"""TRN2 Bass kernel v2 for nn_Blur: upfirdn2d(pad=(2,1)) with separable 4x4
binomial FIR, x (8, 256, 256, 256) f32, depthwise per (n, c) plane.

v2 vs v1:
 * bf16 end-to-end on device (no hi/lo fp32-emulation): the harness gate is
   rel_err < 2e-2; bf16 in + bf16 intermediate + bf16 out lands ~3e-3.
   Halves HBM traffic (32 MiB in + 32 MiB out per core) and halves PE work.
 * single fused [128, 512] PSUM tile per pass (one copy per pass instead of
   two), copies alternated between ScalarE and VectorE.
 * output DRAM layout is partition-major [P, C, 2, W] bf16 so each group
   store is one 16 KiB contiguous run per partition (128 descriptors per
   2 MiB DMA instead of 1024 x 2 KiB); host un-permutes + casts to f32.

Math (as v1): out = T_H^T @ X @ T_W per plane, banded Toeplitz T with band
k1 on diagonals -1..+2 (zero boundary). T_H columns even/odd-permuted so
pass2's PSUM partition p holds output rows (2p, 2p+1).
"""
import numpy as np
import ml_dtypes

import concourse.bacc as bacc
import concourse.mybir as mybir
from concourse.tile import TileContext
from concourse.bass_utils import run_bass_kernel_spmd

N, C, H, W = 8, 256, 256, 256
P = 128          # partition size
NCORES = 8
# band: T[i, i+d] = k1[d+1], d in {-1, 0, 1, 2}
BAND_LO, BAND_HI = -1, 2
# pass2 (T_W, natural order): nonzero column ranges of the two 128-row blocks
BLK_COLS = [(0, P + BAND_HI), (P + BAND_LO, 2 * P)]   # [0,130), [127,256)
# pass1 (T_H, even/odd-permuted cols): nonzero column spans per 128-row block
P1I = [[(0, 65), (128, 193)], [(64, 128), (191, 256)]]

CG = 16          # channels per DMA group

_CACHE = {}


def _factor_kernel(k2: np.ndarray):
    """Rank-1 factorization k2 = kh (x) kw (float64)."""
    k2 = np.asarray(k2, dtype=np.float64)
    u, s, vt = np.linalg.svd(k2)
    kh = u[:, 0] * np.sqrt(s[0])
    kw = vt[0] * np.sqrt(s[0])
    if kh.sum() < 0:
        kh, kw = -kh, -kw
    return kh, kw


def _toeplitz(n: int, k1: np.ndarray) -> np.ndarray:
    """T[i, j] = k1[j - i + 1] for 0 <= j-i+1 < 4, zero elsewhere."""
    t = np.zeros((n, n), dtype=np.float64)
    for d in range(BAND_LO, BAND_HI + 1):
        i = np.arange(max(0, -d), min(n, n - d))
        t[i, i + d] = k1[d + 1]
    return t


def _build(n_ch: int, cg: int = CG, reps: int = 1, bufs: int = 3,
           merge_ranges: bool = True, in_gpsimd: bool = False,
           out_scalar: bool = False):
    """Build + compile the per-core Bass program (SPMD, one core's slice)."""
    nc = bacc.Bacc("TRN2", target_bir_lowering=False)

    bf16 = mybir.dt.bfloat16
    f32 = mybir.dt.float32

    assert n_ch % cg == 0
    ng = n_ch // cg
    # [group][partition][c][hb][w] pre-swizzled bf16 input
    xin = nc.declare_dram_parameter("xin", [ng, P, cg * 2 * W], bf16,
                                    isOutput=False)
    # packed Toeplitz constants: cols = [th0 | th1 | tw0 | tw1]
    tcst = nc.declare_dram_parameter("tcst", [P, 2 * H + 2 * W], bf16,
                                     isOutput=False)
    # partition-major output: h = 2p + s
    out = nc.declare_dram_parameter("out", [P, n_ch, 2, W], bf16,
                                    isOutput=True)

    with TileContext(nc) as tc:
        with (tc.tile_pool(name="const", bufs=1) as cpool,
              tc.tile_pool(name="xin_p", bufs=bufs) as xpool,
              tc.tile_pool(name="mid", bufs=6) as mpool,
              tc.tile_pool(name="zout", bufs=bufs) as zpool,
              tc.tile_pool(name="psy", bufs=4, space="PSUM") as pypool,
              tc.tile_pool(name="psz", bufs=4, space="PSUM") as pzpool):

            tc_tile = cpool.tile([P, 2 * H + 2 * W], bf16, name="tcst",
                                 tag="tcst")
            nc.sync.dma_start(out=tc_tile[:, :], in_=tcst[:, :])

            def tth(b, lo, hi):
                return tc_tile[:, b * H + lo:b * H + hi]

            def ttw(b, lo, hi):
                return tc_tile[:, 2 * H + b * W + lo:2 * H + b * W + hi]

            for g in [gg for _ in range(reps) for gg in range(ng)]:
                # one contiguous 2 MiB load: [128, 16 KiB]
                tx = xpool.tile([P, cg * 2 * W], bf16, name="tx", tag="tx")
                in_eng = nc.gpsimd if in_gpsimd else nc.sync
                in_eng.dma_start(out=tx[:, :], in_=xin[g])

                tz = zpool.tile([P, cg * 2 * W], bf16, name="tz", tag="tz")

                for ci in range(cg):
                    # ---- pass1: Y^T[wb] = sum_hb X[hb,:,wb]^T @ TH[hb]
                    # py cols = (wb, h'): [128, 512] f32 = one PSUM bank
                    tyh = mpool.tile([P, 2 * H], bf16, name="tyh", tag="tyh")
                    py = pypool.tile([P, 2 * H], f32, name="py", tag="py")
                    for wb in range(2):
                        o = wb * H
                        for hb in range(2):
                            off = (ci * 2 + hb) * W + wb * P
                            if hb == 0:
                                ivs = [(0, H)]
                            elif merge_ranges:
                                ivs = [(P1I[1][0][0], P1I[1][1][1])]
                            else:
                                ivs = P1I[1]
                            for ivi, (lo, hi) in enumerate(ivs):
                                nc.tensor.matmul(
                                    py[:, o + lo:o + hi],
                                    tx[:, off:off + P],
                                    tth(hb, lo, hi),
                                    start=(hb == 0),
                                    stop=(hb == 1 and ivi == len(ivs) - 1))
                    if ci % 2 == 0:
                        nc.scalar.copy(tyh[:, :], py[:, :])
                    else:
                        nc.vector.tensor_copy(tyh[:, :], py[:, :])

                    # ---- pass2: Z[s] = sum_wb Y^T[wb,:,s]^T @ TW[wb]
                    # pz cols = (s, w'): partition p -> rows (2p, 2p+1)
                    pz = pzpool.tile([P, 2 * W], f32, name="pz", tag="pz")
                    for s in range(2):
                        o = s * W
                        for wb in range(2):
                            lo, hi = (0, W) if wb == 0 else BLK_COLS[1]
                            ysl = slice(wb * H + s * P, wb * H + s * P + P)
                            nc.tensor.matmul(
                                pz[:, o + lo:o + hi], tyh[:, ysl],
                                ttw(wb, lo, hi),
                                start=(wb == 0), stop=(wb == 1))
                    zsl = slice(ci * 2 * W, (ci + 1) * 2 * W)
                    if ci % 2 == 0:
                        nc.vector.tensor_copy(tz[:, zsl], pz[:, :])
                    else:
                        nc.scalar.copy(tz[:, zsl], pz[:, :])

                # ---- store cg channels: 16 KiB contiguous per partition
                out_eng = nc.scalar if out_scalar else nc.sync
                out_eng.dma_start(
                    out=out[:, g * cg:(g + 1) * cg],
                    in_=tz[:, :].rearrange("p (c s w) -> p c s w", c=cg, s=2))
    nc.compile()
    return nc


def _get_nc(n_ch: int):
    key = (n_ch, CG)
    if key not in _CACHE:
        _CACHE[key] = _build(n_ch)
    return _CACHE[key]


def _perm_evenodd(n: int) -> np.ndarray:
    return np.concatenate([np.arange(0, n, 2), np.arange(1, n, 2)])


def _prep_inputs(x: np.ndarray, k2: np.ndarray, n_ch: int):
    cg = CG
    ng = n_ch // cg
    kh, kw = _factor_kernel(k2)
    th64 = _toeplitz(H, kh)[:, _perm_evenodd(H)]   # permuted columns
    tw64 = _toeplitz(W, kw)
    th = th64.astype(ml_dtypes.bfloat16).reshape(2, P, H)
    tw = tw64.astype(ml_dtypes.bfloat16).reshape(2, P, W)
    # packed constants [P, th0 | th1 | tw0 | tw1]
    tcst = np.ascontiguousarray(np.concatenate(
        [th[0], th[1], tw[0], tw[1]], axis=1))

    xhi = np.asarray(x, dtype=np.float32).astype(ml_dtypes.bfloat16)
    # [n, c, h, w] -> [n, g, c', hb, p, w] -> [n, g, p, (c', hb, w)]
    xhi = xhi.reshape(N, ng, cg, 2, P, W).transpose(0, 1, 4, 2, 3, 5)
    xin = np.ascontiguousarray(xhi).reshape(N, ng, P, cg * 2 * W)

    return [{"xin": xin[i], "tcst": tcst} for i in range(NCORES)]


def _run(x: np.ndarray, k2: np.ndarray, trace: bool = False):
    n_ch = C
    nc = _get_nc(n_ch)
    in_maps = _prep_inputs(x, k2, n_ch)
    r = run_bass_kernel_spmd(nc, in_maps, core_ids=list(range(NCORES)),
                             trace=trace)
    # out [P, n_ch, 2, W] bf16: h = 2p + s
    outs = [
        r.results[i]["out"].transpose(1, 0, 2, 3).reshape(n_ch, H, W)
        .astype(np.float32)
        for i in range(NCORES)
    ]
    return np.stack(outs, axis=0), r


def kernel(x: np.ndarray, kernel: np.ndarray) -> np.ndarray:
    out, _ = _run(x, kernel, trace=False)
    return out


# revision 17
# speedup vs baseline: 1.6923x; 1.6923x over previous
"""TRN2 Bass kernel v2 for nn_Blur: upfirdn2d(pad=(2,1)) with separable 4x4
binomial FIR, x (8, 256, 256, 256) f32, depthwise per (n, c) plane.

v2 vs v1:
 * bf16 end-to-end on device (no hi/lo fp32-emulation): the harness gate is
   rel_err < 2e-2; bf16 in + bf16 intermediate + bf16 out lands ~3e-3.
   Halves HBM traffic (32 MiB in + 32 MiB out per core) and halves PE work.
 * single fused [128, 512] PSUM tile per pass (one copy per pass instead of
   two), copies alternated between ScalarE and VectorE.
 * output DRAM layout is group-major [ng, P, cg, 2, W] bf16 so each group
   store is one fully contiguous 2 MiB block (same pattern as the loads);
   host un-permutes + casts to f32.

Math (as v1): out = T_H^T @ X @ T_W per plane, banded Toeplitz T with band
k1 on diagonals -1..+2 (zero boundary). T_H columns even/odd-permuted so
pass2's PSUM partition p holds output rows (2p, 2p+1).
"""
import numpy as np
import ml_dtypes

import concourse.bacc as bacc
import concourse.mybir as mybir
from concourse.tile import TileContext
from concourse.bass_utils import run_bass_kernel_spmd

N, C, H, W = 8, 256, 256, 256
P = 128          # partition size
NCORES = 8
# band: T[i, i+d] = k1[d+1], d in {-1, 0, 1, 2}
BAND_LO, BAND_HI = -1, 2
# pass2 (T_W, natural order): nonzero column ranges of the two 128-row blocks
BLK_COLS = [(0, P + BAND_HI), (P + BAND_LO, 2 * P)]   # [0,130), [127,256)
# pass1 (T_H, even/odd-permuted cols): nonzero column spans per 128-row block
P1I = [[(0, 65), (128, 193)], [(64, 128), (191, 256)]]

CG = 16          # channels per DMA group

_CACHE = {}


def _factor_kernel(k2: np.ndarray):
    """Rank-1 factorization k2 = kh (x) kw (float64)."""
    k2 = np.asarray(k2, dtype=np.float64)
    u, s, vt = np.linalg.svd(k2)
    kh = u[:, 0] * np.sqrt(s[0])
    kw = vt[0] * np.sqrt(s[0])
    if kh.sum() < 0:
        kh, kw = -kh, -kw
    return kh, kw


def _toeplitz(n: int, k1: np.ndarray) -> np.ndarray:
    """T[i, j] = k1[j - i + 1] for 0 <= j-i+1 < 4, zero elsewhere."""
    t = np.zeros((n, n), dtype=np.float64)
    for d in range(BAND_LO, BAND_HI + 1):
        i = np.arange(max(0, -d), min(n, n - d))
        t[i, i + d] = k1[d + 1]
    return t


def _build(n_ch: int, cg: int = CG, reps: int = 1, bufs: int = 3,
           merge_ranges: bool = True, in_gpsimd: bool = False,
           out_scalar: bool = False):
    """Build + compile the per-core Bass program (SPMD, one core's slice)."""
    nc = bacc.Bacc("TRN2", target_bir_lowering=False)

    bf16 = mybir.dt.bfloat16
    f32 = mybir.dt.float32

    assert n_ch % cg == 0
    ng = n_ch // cg
    # [group][partition][c][hb][w] pre-swizzled bf16 input
    xin = nc.declare_dram_parameter("xin", [ng, P, cg * 2 * W], bf16,
                                    isOutput=False)
    # packed Toeplitz constants: cols = [th0 | th1 | tw0 | tw1]
    tcst = nc.declare_dram_parameter("tcst", [P, 2 * H + 2 * W], bf16,
                                     isOutput=False)
    # group-major output (each group store fully contiguous): h = 2p + s
    out = nc.declare_dram_parameter("out", [ng, P, cg, 2, W], bf16,
                                    isOutput=True)

    with TileContext(nc) as tc:
        with (tc.tile_pool(name="const", bufs=1) as cpool,
              tc.tile_pool(name="xin_p", bufs=bufs) as xpool,
              tc.tile_pool(name="mid", bufs=6) as mpool,
              tc.tile_pool(name="zout", bufs=bufs) as zpool,
              tc.tile_pool(name="psy", bufs=4, space="PSUM") as pypool,
              tc.tile_pool(name="psz", bufs=4, space="PSUM") as pzpool):

            tc_tile = cpool.tile([P, 2 * H + 2 * W], bf16, name="tcst",
                                 tag="tcst")
            nc.sync.dma_start(out=tc_tile[:, :], in_=tcst[:, :])

            def tth(b, lo, hi):
                return tc_tile[:, b * H + lo:b * H + hi]

            def ttw(b, lo, hi):
                return tc_tile[:, 2 * H + b * W + lo:2 * H + b * W + hi]

            for g in [gg for _ in range(reps) for gg in range(ng)]:
                # one contiguous 2 MiB load: [128, 16 KiB]
                tx = xpool.tile([P, cg * 2 * W], bf16, name="tx", tag="tx")
                in_eng = nc.gpsimd if in_gpsimd else nc.sync
                in_eng.dma_start(out=tx[:, :], in_=xin[g])

                tz = zpool.tile([P, cg * 2 * W], bf16, name="tz", tag="tz")

                for ci in range(cg):
                    # ---- pass1: Y^T[wb] = sum_hb X[hb,:,wb]^T @ TH[hb]
                    # py cols = (wb, h'): [128, 512] f32 = one PSUM bank
                    tyh = mpool.tile([P, 2 * H], bf16, name="tyh", tag="tyh")
                    py = pypool.tile([P, 2 * H], f32, name="py", tag="py")
                    for wb in range(2):
                        o = wb * H
                        for hb in range(2):
                            off = (ci * 2 + hb) * W + wb * P
                            if hb == 0:
                                ivs = [(0, H)]
                            elif merge_ranges:
                                ivs = [(P1I[1][0][0], P1I[1][1][1])]
                            else:
                                ivs = P1I[1]
                            for ivi, (lo, hi) in enumerate(ivs):
                                nc.tensor.matmul(
                                    py[:, o + lo:o + hi],
                                    tx[:, off:off + P],
                                    tth(hb, lo, hi),
                                    start=(hb == 0),
                                    stop=(hb == 1 and ivi == len(ivs) - 1))
                    if ci % 2 == 0:
                        nc.scalar.copy(tyh[:, :], py[:, :])
                    else:
                        nc.vector.tensor_copy(tyh[:, :], py[:, :])

                    # ---- pass2: Z[s] = sum_wb Y^T[wb,:,s]^T @ TW[wb]
                    # pz cols = (s, w'): partition p -> rows (2p, 2p+1)
                    pz = pzpool.tile([P, 2 * W], f32, name="pz", tag="pz")
                    for s in range(2):
                        o = s * W
                        for wb in range(2):
                            lo, hi = (0, W) if wb == 0 else BLK_COLS[1]
                            ysl = slice(wb * H + s * P, wb * H + s * P + P)
                            nc.tensor.matmul(
                                pz[:, o + lo:o + hi], tyh[:, ysl],
                                ttw(wb, lo, hi),
                                start=(wb == 0), stop=(wb == 1))
                    zsl = slice(ci * 2 * W, (ci + 1) * 2 * W)
                    if ci % 2 == 0:
                        nc.vector.tensor_copy(tz[:, zsl], pz[:, :])
                    else:
                        nc.scalar.copy(tz[:, zsl], pz[:, :])

                # ---- store cg channels: one contiguous 2 MiB block
                out_eng = nc.scalar if out_scalar else nc.sync
                out_eng.dma_start(
                    out=out[g],
                    in_=tz[:, :].rearrange("p (c s w) -> p c s w", c=cg, s=2))
    nc.compile()
    return nc


def _get_nc(n_ch: int):
    key = (n_ch, CG)
    if key not in _CACHE:
        _CACHE[key] = _build(n_ch)
    return _CACHE[key]


def _perm_evenodd(n: int) -> np.ndarray:
    return np.concatenate([np.arange(0, n, 2), np.arange(1, n, 2)])


def _prep_inputs(x: np.ndarray, k2: np.ndarray, n_ch: int):
    cg = CG
    ng = n_ch // cg
    kh, kw = _factor_kernel(k2)
    th64 = _toeplitz(H, kh)[:, _perm_evenodd(H)]   # permuted columns
    tw64 = _toeplitz(W, kw)
    th = th64.astype(ml_dtypes.bfloat16).reshape(2, P, H)
    tw = tw64.astype(ml_dtypes.bfloat16).reshape(2, P, W)
    # packed constants [P, th0 | th1 | tw0 | tw1]
    tcst = np.ascontiguousarray(np.concatenate(
        [th[0], th[1], tw[0], tw[1]], axis=1))

    xhi = np.asarray(x, dtype=np.float32).astype(ml_dtypes.bfloat16)
    # [n, c, h, w] -> [n, g, c', hb, p, w] -> [n, g, p, (c', hb, w)]
    xhi = xhi.reshape(N, ng, cg, 2, P, W).transpose(0, 1, 4, 2, 3, 5)
    xin = np.ascontiguousarray(xhi).reshape(N, ng, P, cg * 2 * W)

    return [{"xin": xin[i], "tcst": tcst} for i in range(NCORES)]


def _run(x: np.ndarray, k2: np.ndarray, trace: bool = False):
    n_ch = C
    nc = _get_nc(n_ch)
    in_maps = _prep_inputs(x, k2, n_ch)
    r = run_bass_kernel_spmd(nc, in_maps, core_ids=list(range(NCORES)),
                             trace=trace)
    # out [ng, P, cg, 2, W] bf16: c = g*cg + c', h = 2p + s
    outs = [
        r.results[i]["out"].transpose(0, 2, 1, 3, 4).reshape(n_ch, H, W)
        .astype(np.float32)
        for i in range(NCORES)
    ]
    return np.stack(outs, axis=0), r


def kernel(x: np.ndarray, kernel: np.ndarray) -> np.ndarray:
    out, _ = _run(x, kernel, trace=False)
    return out
